# revision 12
# baseline (speedup 1.0000x reference)
"""Trainium2 Bass kernel for a 4-layer LIF spiking net (BPSpikingNet).

Reference semantics (per timestep t, per layer l):
    i = h @ W_l.T + b_l
    v = v - v/tau + i          (tau=2  ->  v = 0.5*v + i)
    s = (v >= 1.0)
    v = (1-s) * v              (hard reset to 0)
    h = s
Output = layer-4 spike train, shape [T=32, B=128, 1000], fp32.

Strategy:
  * Data-parallel over batch: B=128 -> 16 samples per core across 8 cores.
  * Layer-by-layer: layer l's matmul input (spikes of l-1) is fully known
    once l-1's recurrence is done, so each layer is ONE dense GEMM over all
    T*Bs = 512 (t,b) columns (neuron-major / weight-stationary, N=512 moving),
    followed by a 32-step elementwise LIF recurrence on [128, O*16] tiles.
  * bf16 matmuls (spikes are exact in bf16; weight rounding is far below the
    spiking threshold margin), fp32 PSUM accumulate, fp32 recurrence.
  * Recurrence: charge writes the charged potential in-place into the current
    buffer iT[:, t] (2 DVE ops per step on the serial chain), and spikes for
    ALL timesteps are extracted afterwards with a single big is_ge op.
"""

import numpy as np
import ml_dtypes

T = 32
B = 128
NCORES = 8
BS = B // NCORES          # 16 samples per core
COLS = T * BS             # 512 (t,b) columns per core
NIN = 2048
KT = NIN // 128           # 16 k-tiles (all layers have 2048 inputs)
O_LIST = [16, 16, 16, 8]  # output 128-tiles per layer (layer 4 padded 1000->1024)
BOFF = [0, 16, 32, 48]    # bias column offset per layer
NB = sum(O_LIST)          # 56 bias columns

_CACHE = {}

TRACE = False             # set True (from test.py) to capture an NTFF profile
LAST_RESULTS = None       # BassKernelResults of the most recent run
EVICT_ENGINE = "scalar"   # "scalar" (ACT Identity+bias) or "vector" fallback


def _build_nc():
    import concourse.mybir as mybir
    import concourse.tile as tile
    from concourse import bacc

    dt = mybir.dt
    alu = mybir.AluOpType

    nc = bacc.Bacc("TRN2", target_bir_lowering=False, debug=False,
                   num_devices=NCORES)

    x_d = nc.dram_tensor("x", [128, KT, COLS], dt.bfloat16, kind="ExternalInput")
    w_d = [
        nc.dram_tensor(f"w{li}", [O_LIST[li], 128, KT, 128], dt.bfloat16,
                       kind="ExternalInput")
        for li in range(4)
    ]
    b_d = nc.dram_tensor("bias", [128, NB], dt.float32, kind="ExternalInput")
    out_d = nc.dram_tensor("out", [128, T, O_LIST[3], BS], dt.bfloat16,
                           kind="ExternalOutput")

    TH = T // 2           # 16 timesteps per half
    HC = TH * BS          # 256 columns per half

    with tile.TileContext(nc) as tc:
        with (
            tc.tile_pool(name="xp", bufs=1) as xp,
            tc.tile_pool(name="sp", bufs=1) as sp,
            tc.tile_pool(name="ip", bufs=2) as ip,
            tc.tile_pool(name="wp", bufs=6) as wp,
            tc.tile_pool(name="vp", bufs=1) as vp,
            tc.tile_pool(name="bp", bufs=1) as bp,
            tc.tile_pool(name="ps", bufs=4, space="PSUM") as ps,
        ):
            # x in 8 chunks on the gpsimd DMA queue (weights go on sync's),
            # so the first matmul's two dependencies transfer in parallel
            xq = []
            for c in range(8):
                xc = xp.tile([128, 2, COLS], dt.bfloat16, tag=f"x{c}")
                nc.gpsimd.dma_start(xc[:], x_d.ap()[:, 2 * c:2 * c + 2, :])
                xq.append(xc)
            bt = bp.tile([128, NB], dt.float32)
            nc.gpsimd.dma_start(bt[:], b_d.ap())

            # PE warmup: ~60 junk matmuls on a zeroed scratch tile while the
            # first DMAs land, so the HAM clock gate opens (1.2->2.4 GHz)
            # before real work arrives. Results go to a scratch PSUM bank
            # that is never read.
            wu = xp.tile([128, 128], dt.bfloat16, tag="warm")
            nc.vector.memset(wu[:], 0.0)
            wacc = ps.tile([128, 128], dt.float32, tag="wacc")
            for _ in range(60):
                nc.tensor.matmul(wacc[:], wu[:], wu[:], start=True, stop=True)

            its = [None] * 4
            sts = [None] * 4
            vbs = [None] * 4

            def gemm_half(li, h):
                O = O_LIST[li]
                it = its[li]
                for o in range(O):
                    wt = wp.tile([128, KT, 128], dt.bfloat16, tag="wt")
                    if li == 0 and h == 0 and o == 0:
                        # split the very first weight DMA so matmul 0 starts
                        # after half the tile has landed
                        nc.sync.dma_start(wt[:, :KT // 2], w_d[0].ap()[0, :, :KT // 2])
                        nc.sync.dma_start(wt[:, KT // 2:], w_d[0].ap()[0, :, KT // 2:])
                    else:
                        nc.sync.dma_start(wt[:], w_d[li].ap()[o])
                    acc = ps.tile([128, HC], dt.float32, tag="acc")
                    for k in range(KT):
                        if li == 0:
                            rhs = xq[k // 2][:, k % 2, h * HC:(h + 1) * HC]
                        else:
                            rhs = sts[li - 1][:, h * TH:(h + 1) * TH, k, :]
                        nc.tensor.matmul(acc[:], wt[:, k, :], rhs,
                                         start=(k == 0), stop=(k == KT - 1))
                    # PSUM -> SBUF eviction with bias add, scattered to t-major
                    bias_ap = bt[:, BOFF[li] + o:BOFF[li] + o + 1]
                    src = acc.rearrange("p (t b) -> p t b", t=TH)
                    dst = it[:, h * TH:(h + 1) * TH, o, :]
                    if EVICT_ENGINE == "scalar":
                        nc.scalar.activation(
                            dst, src, mybir.ActivationFunctionType.Identity,
                            bias=bias_ap, scale=1.0)
                    else:
                        nc.vector.tensor_scalar(dst, src, bias_ap, None, alu.add)

            def rec_half(li, h):
                # charge in place (iT[:,t] becomes the charged potential v(t));
                # only the reset state vb carries between steps
                it, vb = its[li], vbs[li]
                for t in range(h * TH, (h + 1) * TH):
                    nc.vector.scalar_tensor_tensor(
                        it[:, t], vb[:], 0.5, it[:, t], alu.mult, alu.add)
                    nc.vector.scalar_tensor_tensor(
                        vb[:], it[:, t], 1.0, it[:, t], alu.is_lt, alu.mult)
                    if li == 3 and t == h * TH + TH // 2 - 1:
                        # output layer: extract+ship the finished quarter while
                        # the chain continues, so the tail only waits on 8 steps
                        ql = slice(h * TH, t + 1)
                        nc.vector.tensor_scalar(
                            sts[3][:, ql], it[:, ql], 1.0, None, alu.is_ge)
                        nc.sync.dma_start(out_d.ap()[:, ql], sts[3][:, ql])
                if li == 3:
                    ql = slice(h * TH + TH // 2, (h + 1) * TH)
                    nc.vector.tensor_scalar(
                        sts[3][:, ql], it[:, ql], 1.0, None, alu.is_ge)
                    nc.sync.dma_start(out_d.ap()[:, ql], sts[3][:, ql])
                else:
                    sl = slice(h * TH, (h + 1) * TH)
                    nc.vector.tensor_scalar(
                        sts[li][:, sl], it[:, sl], 1.0, None, alu.is_ge)

            for li in range(4):
                O = O_LIST[li]
                its[li] = ip.tile([128, T, O, BS], dt.float32, tag="it",
                                  name=f"it{li}")
                sts[li] = sp.tile([128, T, O, BS], dt.bfloat16, tag=f"s{li}",
                                  name=f"s{li}")
                vbs[li] = vp.tile([128, O, BS], dt.float32, tag=f"vb{li}",
                                  name=f"vb{li}")
                nc.vector.memset(vbs[li][:], 0.0)
                # pipeline: gemm(li,h1); gemm(li,h2) || rec(li,h1);
                # next layer's gemm h1 || rec(li,h2)
                gemm_half(li, 0)
                gemm_half(li, 1)
                rec_half(li, 0)
                rec_half(li, 1)

    nc.compile()
    return nc


def _get_nc():
    if "nc" not in _CACHE:
        _CACHE["nc"] = _build_nc()
    return _CACHE["nc"]


def _host_inputs(x_tbf, Ws, bs):
    """Shared (weight/bias) arrays + per-core x shards, pre-laid-out."""
    bf16 = ml_dtypes.bfloat16
    w_arrs = []
    b_cols = []
    for li in range(4):
        W = np.asarray(Ws[li], np.float32)
        b = np.asarray(bs[li], np.float32)
        O = O_LIST[li]
        if W.shape[0] < O * 128:           # pad layer 4: 1000 -> 1024
            pad = O * 128 - W.shape[0]
            W = np.concatenate([W, np.zeros((pad, NIN), np.float32)], 0)
            b = np.concatenate([b, np.zeros(pad, np.float32)])
        # warr[o, ki, k, mo] = W[o*128+mo, k*128+ki]
        w_arrs.append(np.ascontiguousarray(
            W.reshape(O, 128, KT, 128).transpose(0, 3, 2, 1)).astype(bf16))
        b_cols.append(b.reshape(O, 128))
    b_all = np.ascontiguousarray(np.concatenate(b_cols, 0).T).astype(np.float32)

    x = np.asarray(x_tbf, np.float32)
    x_shards = []
    for c in range(NCORES):
        xc = x[:, c * BS:(c + 1) * BS, :]                    # [T, BS, NIN]
        xc = xc.transpose(2, 0, 1).reshape(NIN, COLS)        # [n, t*BS+b]
        xc = xc.reshape(KT, 128, COLS).transpose(1, 0, 2)    # [p, k, cols]
        x_shards.append(np.ascontiguousarray(xc).astype(bf16))
    return w_arrs, b_all, x_shards


def _decode_out(oc):
    """[128, T, 8, BS] (p,t,o,b) -> [T, BS, 1000] fp32."""
    oc = np.asarray(oc).astype(np.float32)
    oc = oc.transpose(1, 3, 2, 0).reshape(T, BS, O_LIST[3] * 128)
    return oc[:, :, :1000]


def kernel(x_tbf, W1, b1, W2, b2, W3, b3, W4, b4):
    global LAST_RESULTS
    from concourse.bass_utils import run_bass_kernel_spmd

    nc = _get_nc()
    w_arrs, b_all, x_shards = _host_inputs(
        x_tbf, [W1, W2, W3, W4], [b1, b2, b3, b4])

    in_maps = []
    for c in range(NCORES):
        m = {"x": x_shards[c], "bias": b_all}
        for li in range(4):
            m[f"w{li}"] = w_arrs[li]
        in_maps.append(m)

    res = run_bass_kernel_spmd(nc, in_maps, core_ids=list(range(NCORES)),
                               trace=TRACE)
    LAST_RESULTS = res

    out = np.empty((T, B, 1000), np.float32)
    for c in range(NCORES):
        out[:, c * BS:(c + 1) * BS, :] = _decode_out(res.results[c]["out"])
    return out
